# revision 12
# baseline (speedup 1.0000x reference)
"""DetectorLoss on 8 Trainium2 NeuronCores (Bass/Tile) — v3.

Strategy (data-parallel over batch, 4 images per core):
  * Host re-layouts (pure transpose/concat, no data-dependent indexing) the
    per-cell sparse data into 25-f32 records [d0..d3, c0..c19, po]; each
    positive is ONE contiguous 100B indirect fetch.  Positives at adjacent
    cells (x, x+1) are pair-merged into a single 200B fetch, so the sparse
    fetch is 3 SWDGE instructions (128 descriptors each) writing into one
    [128, K*25] tile; d/po are strided views (free), the class value is a
    host one-hot mask multiply + reduce.
  * pred_obj streamed as bf16 (2 chunks); background SmoothL1 sum via
    sum sl1 = 0.5*(QS - RD): QS = sum x^2 (ACT Square+accum), RD = sum r^2
    with r = max(|x|,1)-1 (DVE) squared+accumulated on ACT.
  * Single ACT table set (natural_log_exp_and_others) forced via a
    get_activation_tables patch - no mid-kernel ACT_TABLE_LOADs.
  * The post-gather critical chain (tanh -> box -> SIoU -> obj corr) runs
    on DVE with ACT only for Exp/Square; algebraic folds keep it short:
    siou == B-1 with B = iou + 0.5*egs - 0.5*shs (constants folded into
    the host-side combine), sl1 via t=-0.5*min(|v|,1) fused min+mult.
  * Host combines per-core partial sums (weighted means).
"""
import numpy as np

B, A, C, H, W = 32, 3, 20, 160, 160
HW = H * W
M = 8            # cores
Bm = B // M      # images per core
NCELL = Bm * A * HW           # 307200 cells per core
SZ_OBJ = NCELL
REC = 25                      # [d0..d3, c0..c19, po]
NCHUNK = 2
F = (SZ_OBJ // 128) // NCHUNK  # 1200
EPS = 1e-7
NCOLS = NCHUNK + 4            # QS*2, RD, jk1, jk2, jk3

_NC_CACHE = {}


def _patch_act_tables():
    """Force every activation onto the natural_log_exp_and_others set so the
    whole kernel needs exactly one ACT_TABLE_LOAD (it has ln+exp+square)."""
    import concourse.bacc as bacc_mod
    from concourse.hw_specs import get_activation_tables as orig

    if getattr(bacc_mod.get_activation_tables, "_detloss_patch", False):
        return

    def patched(arch):
        t = orig(arch)
        return {name: (fns if name == "natural_log_exp_and_others" else set())
                for name, fns in t.items()}

    patched._detloss_patch = True
    bacc_mod.get_activation_tables = patched


def _build_nc(K, spans):
    """spans: tuple of (col_start, n_records) per indirect-DMA instruction;
    each fetches n_records*25 contiguous f32 per partition into X."""
    import concourse.bass as bass
    import concourse.bacc as bacc
    import concourse.tile as tile
    from concourse import mybir

    _patch_act_tables()

    f32 = mybir.dt.float32
    bf16 = mybir.dt.bfloat16
    i32 = mybir.dt.int32
    op = mybir.AluOpType
    act = mybir.ActivationFunctionType
    NI = len(spans)

    nc = bacc.Bacc("TRN2", target_bir_lowering=False, debug=False)
    obj16_p = nc.dram_tensor("obj16", [SZ_OBJ, 1], bf16, kind="ExternalInput")
    xrec_p = nc.dram_tensor("xrec", [NCELL * REC, 1], f32, kind="ExternalInput")
    ioffs_p = nc.dram_tensor("ioffs", [128, NI], i32, kind="ExternalInput")
    NHD = 12 * K + 25 * K
    hd_p = nc.dram_tensor("hd", [128, NHD], f32, kind="ExternalInput")
    out_p = nc.dram_tensor("partials", [128, NCOLS], f32, kind="ExternalOutput")

    with tile.TileContext(nc) as tc, \
         tc.tile_pool(name="io", bufs=1) as io, \
         tc.tile_pool(name="wk", bufs=1) as wk, \
         tc.tile_pool(name="st", bufs=NCHUNK + 1) as st:
        ioffs = io.tile([128, NI], i32)
        hd = io.tile([128, NHD], f32)
        partials = io.tile([128, NCOLS], f32)
        nc.sync.dma_start(out=ioffs[:, :], in_=ioffs_p[:, :])
        nc.sync.dma_start(out=hd[:, :], in_=hd_p[:, :])

        # single ACT table set; warm it at t=0
        warm = wk.tile([128, 1], f32, name="warm", tag="warm")
        nc.vector.memset(warm[:, :], 1.0)
        warm2 = wk.tile([128, 1], f32, name="warm2", tag="warm2")
        nc.scalar.activation(out=warm2[:, :], in_=warm[:, :], func=act.Exp)

        # ---- sparse fetch: 25-f32 records, pair-merged spans ----
        X = wk.tile([128, K, REC], f32, name="X", tag="X")
        Xf = bass.AP(tensor=X[:, :, :].tensor, offset=X[:, :, :].offset,
                     ap=[X[:, :, :].ap[0], [1, K * REC]])
        for i, (col, nrec) in enumerate(spans):
            dst = bass.AP(tensor=Xf.tensor, offset=Xf.offset + col * REC,
                          ap=[Xf.ap[0], [1, nrec * REC]])
            nc.gpsimd.indirect_dma_start(
                out=dst, out_offset=None, in_=xrec_p[:, :],
                in_offset=bass.IndirectOffsetOnAxis(
                    ap=ioffs[:, i:i + 1], axis=0))

        # ---- streamed background pass over pred_obj (bf16) ----
        rbig = wk.tile([128, NCHUNK * F], bf16, name="rbig", tag="rbig")
        for c in range(NCHUNK):
            x = st.tile([128, F], bf16)
            chunk = bass.AP(tensor=obj16_p[:, :].tensor, offset=c * 128 * F,
                            ap=[[F, 128], [1, F]])
            nc.sync.dma_start(out=x[:, :], in_=chunk)
            q = st.tile([128, F], bf16)
            nc.scalar.activation(out=q[:, :], in_=x[:, :], func=act.Square,
                                 accum_out=partials[:, c:c + 1])
            ab = st.tile([128, F], bf16)
            nc.vector.scalar_tensor_tensor(out=ab[:, :], in0=x[:, :],
                                           scalar=-1.0, in1=x[:, :],
                                           op0=op.mult, op1=op.max)
            nc.vector.tensor_scalar(out=rbig[:, c * F:(c + 1) * F],
                                    in0=ab[:, :], scalar1=1.0, scalar2=-1.0,
                                    op0=op.max, op1=op.add)

        # ---- per-positive math ----
        PR = [128, 2 * K]
        SG = [128, K]
        import itertools
        _cnt = itertools.count()

        def mk(shape):
            n = f"t{next(_cnt)}"
            return wk.tile(shape, f32, name=n, tag=n)

        def pr():
            return mk(PR)

        def sg():
            return mk(SG)

        pxy1 = hd[:, 0:2 * K]          # 1 + p_xy
        anc = hd[:, 2 * K:4 * K]       # anchors (w,h)
        gtc = hd[:, 4 * K:6 * K]       # gt box center
        gtwh = hd[:, 6 * K:8 * K]      # gt box w,h
        facw2 = hd[:, 8 * K:9 * K]     # -2 * win * 0.25*HW / n_img
        w15 = hd[:, 9 * K:10 * K]      # 1.5 * win
        valid = hd[:, 10 * K:11 * K]   # 1 for real lanes
        padb = hd[:, 11 * K:12 * K]    # 1 for pad lanes
        clsmask = bass.AP(tensor=hd[:, :].tensor, offset=hd[:, :].offset + 12 * K,
                          ap=[hd[:, :].ap[0], [25, K], [1, 25]])  # [128,K,25]

        xv = X[:, :, :]

        def xview(off, nplane):
            return bass.AP(tensor=xv.tensor, offset=xv.offset + off,
                           ap=[xv.ap[0], [1, nplane], [REC, K]])

        d01 = xview(0, 2)     # [128, 2, K] plane-major d0|d1
        d23 = xview(2, 2)
        po = bass.AP(tensor=xv.tensor, offset=xv.offset + 24,
                     ap=[xv.ap[0], [REC, K]])            # [128, K]
        x25 = bass.AP(tensor=xv.tensor, offset=xv.offset,
                      ap=[xv.ap[0], [REC, K], [1, 25]])  # [128, K, 25]

        V = nc.vector
        P_ = nc.gpsimd
        A_ = nc.scalar.activation
        tt = lambda e, o, a, b, alu: e.tensor_tensor(out=o, in0=a, in1=b, op=alu)
        ts = lambda e, o, a, s1, s2, o0, o1=None, acc=None: e.tensor_scalar(
            out=o, in0=a, scalar1=s1, scalar2=s2, op0=o0,
            **({"op1": o1} if o1 is not None else {}),
            **({"accum_out": acc} if acc is not None else {}))
        stt = lambda e, o, a, s, b, o0, o1, acc=None: e.scalar_tensor_tensor(
            out=o, in0=a, scalar=s, in1=b, op0=o0, op1=o1,
            **({"accum_out": acc} if acc is not None else {}))

        # gt-side prep (gather-independent)
        epsv = wk.tile(PR, f32, name="epsv", tag="epsv")
        nc.gpsimd.memset(epsv[:, 0:K], 0.0)
        nc.gpsimd.memset(epsv[:, K:2 * K], EPS)
        b2lo = pr(); stt(V, b2lo[:], gtwh, -0.5, gtc, op.mult, op.add)
        b2hi = pr(); stt(V, b2hi[:], gtwh, 0.5, gtc, op.mult, op.add)
        whe2b = pr(); tt(P_, whe2b[:], gtwh, epsv[:], op.add)
        area2 = sg(); tt(P_, area2[:], whe2b[:, 0:K], whe2b[:, K:2 * K], op.mult)
        s1 = pr(); tt(V, s1[:], b2lo[:], b2hi[:], op.add)

        # tanh(d01) + pxy (exp-based), pred box.  Wait tiers order the ACT
        # queue: QS0, QS1, e2, ex, RD, e4s, sq1, e4, lnp.
        ctx_a = tc.tile_wait_until(0.0095)
        ctx_a.__enter__()
        e2 = pr(); A_(out=e2[:], in_=d01, func=act.Exp, scale=2.0)
        e2p = pr(); ts(V, e2p[:], e2[:], 1.0, None, op.add)
        re2 = pr(); V.reciprocal(out=re2[:], in_=e2p[:])
        c1 = pr(); stt(V, c1[:], re2[:], -2.0, pxy1, op.mult, op.add)
        ex = pr(); A_(out=ex[:], in_=d23, func=act.Exp)
        ctx_a.__exit__(None, None, None)
        ctx_b = tc.tile_wait_until(0.0105)
        ctx_b.__enter__()
        r2big = wk.tile([128, NCHUNK * F], bf16, name="r2big", tag="r2big")
        nc.scalar.activation(out=r2big[:, :], in_=rbig[:, :], func=act.Square,
                             accum_out=partials[:, NCHUNK:NCHUNK + 1])
        ctx_b.__exit__(None, None, None)
        tail_ctx = tc.tile_wait_until(0.011)
        tail_ctx.__enter__()
        wh1 = pr(); stt(V, wh1[:], ex[:], float(W), anc, op.mult, op.mult)
        # non-critical strands on Pool (add/sub/mult only)
        po1v = sg(); ts(P_, po1v[:], po, 1.0, None, op.add)      # po+1
        clsm = mk([128, K, 25]); tt(P_, clsm[:], clsmask, x25, op.mult)
        whe1b = pr(); tt(P_, whe1b[:], wh1[:], epsv[:], op.add)
        area1 = sg(); tt(P_, area1[:], whe1b[:, 0:K], whe1b[:, K:2 * K], op.mult)
        u1 = sg(); tt(P_, u1[:], area1[:], area2[:], op.add)
        u1e = sg(); ts(P_, u1e[:], u1[:], EPS, None, op.add)
        wd = pr(); tt(P_, wd[:], whe1b[:], whe2b[:], op.subtract)
        wda = pr(); stt(V, wda[:], wd[:], -1.0, wd[:], op.mult, op.max)
        mxw = pr(); tt(V, mxw[:], whe1b[:], whe2b[:], op.max)
        rmx = pr(); V.reciprocal(out=rmx[:], in_=mxw[:])

        b1lo = pr(); stt(V, b1lo[:], wh1[:], -0.5, c1[:], op.mult, op.add)
        b1hi = pr(); stt(V, b1hi[:], wh1[:], 0.5, c1[:], op.mult, op.add)
        # angle chain heads straight off c1: b1lo+b1hi == 2*c1
        sdf = pr(); stt(V, sdf[:], c1[:], -2.0, s1[:], op.mult, op.add)
        prodf = sg(); tt(V, prodf[:], sdf[:, 0:K], sdf[:, K:2 * K], op.mult)
        sqsf = pr(); tt(V, sqsf[:], sdf[:], sdf[:], op.mult)
        sig2f = sg(); tt(V, sig2f[:], sqsf[:, 0:K], sqsf[:, K:2 * K], op.add)
        rsig2 = sg(); V.reciprocal(out=rsig2[:], in_=sig2f[:])
        aprodf = sg(); stt(V, aprodf[:], prodf[:], -1.0, prodf[:], op.mult, op.max)
        angle = sg(); stt(V, angle[:], aprodf[:], 2.0, rsig2[:], op.mult, op.mult)
        gamma = sg(); ts(V, gamma[:], angle[:], -2.0, None, op.add)
        # iou chain
        mnhi = pr(); tt(V, mnhi[:], b1hi[:], b2hi[:], op.min)
        mxlo = pr(); tt(V, mxlo[:], b1lo[:], b2lo[:], op.max)
        itax = pr(); tt(V, itax[:], mnhi[:], mxlo[:], op.subtract)
        itax2 = pr(); ts(V, itax2[:], itax[:], 0.0, None, op.max)
        inter = sg(); tt(V, inter[:], itax2[:, 0:K], itax2[:, K:2 * K], op.mult)
        u2 = sg(); stt(V, u2[:], inter[:], -1.0, u1e[:], op.mult, op.add)
        ru = sg(); V.reciprocal(out=ru[:], in_=u2[:])
        iou = sg(); tt(V, iou[:], inter[:], ru[:], op.mult)
        # rho chain
        cwmax = pr(); tt(V, cwmax[:], b1hi[:], b2hi[:], op.max)
        cwmin = pr(); tt(V, cwmin[:], b1lo[:], b2lo[:], op.min)
        cw = pr(); tt(V, cw[:], cwmax[:], cwmin[:], op.subtract)
        rcw = pr(); V.reciprocal(out=rcw[:], in_=cw[:])
        srw = pr(); stt(V, srw[:], sdf[:], 0.5, rcw[:], op.mult, op.mult)
        rho = pr(); tt(V, rho[:], srw[:], srw[:], op.mult)
        # shape-cost branch first: its exp input (wda*rmx) is ready early,
        # so sq1/sh/shs complete before the dist branch reaches e4ab
        grhoC = pr(); stt(V, grhoC[:], wda[:], -1.0, rmx[:], op.mult, op.mult)
        e4s = pr(); A_(out=e4s[:], in_=grhoC[:], func=act.Exp)
        sq1 = pr(); A_(out=sq1[:], in_=e4s[:], func=act.Square,
                       scale=-1.0, bias=1.0)
        sh = pr(); tt(V, sh[:], sq1[:], sq1[:], op.mult)
        shs = sg(); tt(V, shs[:], sh[:, 0:K], sh[:, K:2 * K], op.add)
        grho4 = wk.tile([128, 2 * K], f32, name="grho4", tag="grho4")
        tt(V, grho4[:, 0:K], gamma[:], rho[:, 0:K], op.mult)
        tt(V, grho4[:, K:2 * K], gamma[:], rho[:, K:2 * K], op.mult)
        e4 = wk.tile([128, 2 * K], f32, name="e4", tag="e4")
        A_(out=e4[:, :], in_=grho4[:, :], func=act.Exp)
        egs = sg(); tt(V, egs[:], e4[:, 0:K], e4[:, K:2 * K], op.add)
        # siou == B-1, B = iou + 0.5*egs - 0.5*shs
        Av = sg(); stt(V, Av[:], egs[:], 0.5, iou[:], op.mult, op.add)
        Bv = sg(); stt(V, Bv[:], shs[:], -0.5, Av[:], op.mult, op.add)



        # obj correction: sl1(v) = -2*t*(|v|+t), t = -0.5*min(|v|,1) (fused);
        # the -2/0.75 factors live in facw2/w15.
        ctx_c = tc.tile_wait_until(0.0113)
        ctx_c.__enter__()
        a2 = sg(); stt(V, a2[:], po, -1.0, po, op.mult, op.max)  # |po|
        tm2 = sg(); ts(V, tm2[:], a2[:], 1.0, -0.5, op.min, op.mult)
        u2t = sg(); tt(P_, u2t[:], tm2[:], a2[:], op.add)
        termB = sg(); tt(P_, termB[:], tm2[:], u2t[:], op.mult)
        termB2 = sg(); tt(P_, termB2[:], w15, termB[:], op.mult)
        ctx_c.__exit__(None, None, None)
        dif = sg(); stt(V, dif[:], Bv[:], -1.0, po1v[:], op.mult, op.add)
        ad = sg(); stt(V, ad[:], dif[:], -1.0, dif[:], op.mult, op.max)
        tmd = sg(); ts(V, tmd[:], ad[:], 1.0, -0.5, op.min, op.mult)
        ud = sg(); tt(V, ud[:], tmd[:], ad[:], op.add)
        sdt = sg(); tt(V, sdt[:], tmd[:], ud[:], op.mult)
        x1 = sg(); tt(V, x1[:], facw2, sdt[:], op.mult)
        jk3 = sg(); stt(V, jk3[:], termB2[:], 1.0, x1[:], op.mult, op.add,
                        acc=partials[:, NCHUNK + 3:NCHUNK + 4])
        jk1 = sg(); stt(V, jk1[:], Bv[:], -1.0, valid, op.mult, op.mult,
                        acc=partials[:, NCHUNK + 1:NCHUNK + 2])
        tail_ctx.__exit__(None, None, None)
        ctx_d = tc.tile_wait_until(0.0115)
        ctx_d.__enter__()
        from concourse import mybir as _mb
        pcg = sg(); V.tensor_reduce(out=pcg[:], in_=clsm[:], op=op.add,
                                    axis=_mb.AxisListType.X)
        pcgb = sg(); tt(P_, pcgb[:], pcg[:], padb, op.add)
        lnp = sg(); A_(out=lnp[:], in_=pcgb[:], func=act.Ln)
        jk2 = sg(); ts(V, jk2[:], lnp[:], -1.0, 0.0, op.mult, op.add,
                       acc=partials[:, NCHUNK + 2:NCHUNK + 3])
        ctx_d.__exit__(None, None, None)

        nc.sync.dma_start(out=out_p[:, :], in_=partials[:, :])

    return nc


def _get_nc(K, spans, finalized=True):
    key = (K, spans, finalized)
    if key not in _NC_CACHE:
        nc = _build_nc(K, spans)
        if finalized:
            nc.finalize()
        else:
            nc.compile()
        _NC_CACHE[key] = nc
    return _NC_CACHE[key]


def _pack(vals, K, fill, dtype):
    """lane j = i*128 + p  ->  tile[p, i]."""
    out = np.full((K, 128), fill, dtype)
    out.reshape(-1)[:len(vals)] = vals
    return out.T


def _pair_order(cell):
    """Greedy pair-merge of adjacent cells. Returns (order, npair) where
    order lists positive indices as [pair_firsts | pair_seconds | singles]
    with len(pair_firsts) == len(pair_seconds) == npair <= 128."""
    ns = len(cell)
    srt = np.argsort(cell, kind="stable")
    cs = cell[srt]
    firsts, seconds, singles = [], [], []
    i = 0
    while i < ns:
        if i + 1 < ns and cs[i + 1] == cs[i] + 1 and len(firsts) < 128:
            firsts.append(srt[i]); seconds.append(srt[i + 1])
            i += 2
        else:
            singles.append(srt[i])
            i += 1
    # cell-sorted order is preserved within each group -> consecutive
    # descriptors hit nearby DRAM addresses
    return np.array(firsts + seconds + singles, np.int64), len(firsts)


def host_prep(pred_obj, pred_delta_box, pred_cls, gt_box, gt_cls,
              p_batch_idx, p_x_idx, p_y_idx, p_anchor_idx, anchors):
    """Shard inputs; integer index prep + pure re-layouts.
    Returns (in_maps, K, spans, P)."""
    import ml_dtypes
    f32 = np.float32
    pred_obj = np.asarray(pred_obj, f32)
    pred_delta_box = np.asarray(pred_delta_box, f32)
    pred_cls = np.asarray(pred_cls, f32)
    gt_box = np.asarray(gt_box, f32)
    gt_cls = np.asarray(gt_cls, np.int64)
    p_b = np.asarray(p_batch_idx, np.int64)
    p_x = np.asarray(p_x_idx, np.int64)
    p_y = np.asarray(p_y_idx, np.int64)
    p_a = np.asarray(p_anchor_idx, np.int64)
    anchors = np.asarray(anchors, f32)
    P = len(p_b)

    n_img = np.bincount(p_b, minlength=B)
    cellg = ((p_b * H + p_y) * W + p_x) * A + p_a
    win = np.zeros(P, f32)
    _, ridx = np.unique(cellg[::-1], return_index=True)
    win[P - 1 - ridx] = 1.0

    core_of = p_b // Bm

    # per-core pair-merge: lanes [pairs_first | pairs_second | singles]
    orders, npairs = [], []
    for m in range(M):
        sel = np.nonzero(core_of == m)[0]
        bl = p_b[sel] - m * Bm
        cell = (bl * A + p_a[sel]) * HW + p_y[sel] * W + p_x[sel]
        order, npair = _pair_order(cell)
        orders.append(sel[order])
        npairs.append(npair)

    max_sing = max(len(o) - 2 * n for o, n in zip(orders, npairs))
    if any(npairs) and max_sing <= 2 * 128:
        nsing_cols = max(1, -(-max_sing // 128))
        K = 2 + nsing_cols
        spans = ((0, 2),) + tuple((2 + i, 1) for i in range(nsing_cols))
    else:
        counts = np.bincount(core_of, minlength=M)
        K = max(1, -(-int(counts.max()) // 128))
        spans = tuple((i, 1) for i in range(K))
        orders = [np.nonzero(core_of == m)[0] for m in range(M)]
        npairs = [0] * M

    in_maps = []
    for m in range(M):
        o = orders[m]
        npair = npairs[m]
        ns = len(o)
        bl = p_b[o] - m * Bm
        xj, yj, aj, cj = p_x[o], p_y[o], p_a[o], gt_cls[o]
        cell = (bl * A + aj) * HW + yj * W + xj

        # lane j = i*128 + p; pair firsts -> col 0, seconds -> col 1,
        # singles -> cols 2.. (or cols 0.. in the no-pairs layout)
        lane_of = np.empty(ns, np.int64)
        if spans[0][1] == 2:
            lane_of[:npair] = np.arange(npair)
            lane_of[npair:2 * npair] = 128 + np.arange(npair)
            lane_of[2 * npair:] = 2 * 128 + np.arange(ns - 2 * npair)
        else:
            lane_of[:] = np.arange(ns)
        nlanes = K * 128

        def lanes(vals, fill):
            outv = np.full(nlanes, fill, f32)
            outv[lane_of] = vals.astype(f32)
            return _pack(outv, K, fill, f32)

        gtb = gt_box[o]
        ancg = anchors[aj]
        winm = win[o]
        fac = winm * (0.25 * HW) / n_img[p_b[o]]
        vmask = np.ones(ns, f32)
        hd_planes = [
            lanes(1.0 + xj, 1.0), lanes(1.0 + yj, 1.0),
            lanes(ancg[:, 0], 0.1), lanes(ancg[:, 1], 0.1),
            lanes(gtb[:, 0], 0.5), lanes(gtb[:, 1], 0.5),
            lanes(gtb[:, 2], 0.5), lanes(gtb[:, 3], 0.5),
            lanes(-2.0 * fac, 0.0), lanes(1.5 * winm, 0.0),
            lanes(vmask, 0.0), 1.0 - lanes(vmask, 0.0),
        ]
        cm = np.zeros((nlanes, 25), f32)
        cm[lane_of, 4 + cj] = 1.0
        cmt = cm.reshape(K, 128, 25).transpose(1, 0, 2).reshape(128, K * 25)
        hd = np.concatenate(hd_planes + [cmt], axis=1)

        # per-span offset columns (offset of the span's first record)
        cell_of_lane = np.zeros(nlanes, np.int64)
        cell_of_lane[lane_of] = cell
        offcols = []
        for col, nrec in spans:
            if nrec == 2:
                oc = cell_of_lane[0:128]        # pair firsts (col 0 lanes)
            else:
                oc = cell_of_lane[col * 128:(col + 1) * 128]
            offcols.append((oc * REC).astype(np.int32))
        ioffs = np.stack(offcols, axis=1)

        sl = slice(m * Bm, (m + 1) * Bm)
        xr = np.empty((NCELL, REC), f32)
        xr[:, 0:4] = pred_delta_box[sl].transpose(0, 1, 3, 4, 2).reshape(-1, 4)
        xr[:, 4:24] = pred_cls[sl].transpose(0, 1, 3, 4, 2).reshape(-1, 20)
        xr[:, 24] = pred_obj[sl].reshape(-1)

        in_maps.append({
            "obj16": np.ascontiguousarray(
                pred_obj[sl].reshape(-1, 1).astype(ml_dtypes.bfloat16)),
            "xrec": np.ascontiguousarray(xr.reshape(-1, 1)),
            "ioffs": np.ascontiguousarray(ioffs),
            "hd": np.ascontiguousarray(hd),
        })
    return in_maps, K, spans, P


def combine(partials_list, P):
    """Host reduction of per-core [128, NCOLS] partial sums."""
    tot_QS = tot_RD = tot_j1 = tot_j2 = tot_j3 = 0.0
    for pt in partials_list:
        pt = np.asarray(pt, np.float64)
        tot_QS += pt[:, :NCHUNK].sum()
        tot_RD += pt[:, NCHUNK].sum()
        tot_j1 += pt[:, NCHUNK + 1].sum()
        tot_j2 += pt[:, NCHUNK + 2].sum()
        tot_j3 += pt[:, NCHUNK + 3].sum()
    iou_loss = (2 * P + tot_j1) / P      # jk1 = -sum B*valid; siou = B-1
    cls_loss = tot_j2 / P
    obj_loss = (0.375 * (tot_QS - tot_RD) + tot_j3) / (B * A * H * W)
    tot_loss = iou_loss + 4 * obj_loss + 2 * cls_loss
    return (np.float32(iou_loss), np.float32(obj_loss),
            np.float32(cls_loss), np.float32(tot_loss))


def kernel(pred_obj, pred_delta_box, pred_cls, gt_box, gt_cls,
           p_batch_idx, p_x_idx, p_y_idx, p_anchor_idx, anchors):
    from concourse.bass_utils import run_bass_kernel_spmd
    in_maps, K, spans, P = host_prep(pred_obj, pred_delta_box, pred_cls,
                                     gt_box, gt_cls, p_batch_idx, p_x_idx,
                                     p_y_idx, p_anchor_idx, anchors)
    nc = _get_nc(K, spans)
    res = run_bass_kernel_spmd(nc, in_maps, list(range(M))).results
    return combine([r["partials"] for r in res], P)
